# revision 7
# baseline (speedup 1.0000x reference)
"""ListFoldLoss Trainium2 kernel (8-core SPMD, Bass/Tile).

Math: the reference builds D[u,v] = exp(f_u - f_v) (rank-1: exp(f) x exp(-f))
and takes nested-window sums of it.  Every window sum factorizes:

    S(i) = A(i) * B(i),   A(i) = sum_{rank in [i, N-i)} exp(pred),
                          B(i) = sum_{rank in [i, N-i)} exp(-pred)

so the NxN matrix never needs to exist.  With r_u = rank of element u when
sorting by target descending and m_u = min(r_u, N-1-r_u):

    A(i)  = sum_u exp(pred_u)  * [m_u >= i]
    B(i)  = sum_u exp(-pred_u) * [m_u >= i]
    num   = sum_u pred_u * (2*[r_u < N/2] - 1)        (= sum_i log num_i)
    loss  = sum_i log(A(i)*B(i) - (N-2i)) - num

Two-level window sums: write i = 32*Q + S (Q in [0,128), S in [0,32)) and
q_u = floor(m_u/32), s_u = m_u mod 32.  Then [m >= i] = [q > Q] + [q==Q][s>=S]:

    A(32Q+S) = SufH_a[Q] + F_a[Q,S]
    H_a[Q]   = sum_u a_u [q_u == Q]          (coarse histogram)
    SufH_a[Q]= sum_{Q' > Q} H_a[Q']          (strict suffix, one tri-matmul)
    F_a[Q,S] = sum_u a_u [q_u == Q][s_u >= S]

H/F are built from per-core partial histograms over each core's own 1024
elements (tiny per-subtile masks + PE matmuls into PSUM) and combined with a
single AllReduce of [128, 67] f32 (64 F cols + 2 H cols + num partial).
Every core then computes the identical full loss; the host reads core 0.

Device plan (per core c of 8):
  phase 1: ranks r_u for u in the core's 1024-slice, via N comparisons per u:
           DVE fused tensor_scalar(is_gt)+accum_out for 5 of 8 u-subtiles and
           ACT fused sign(t_j - t_u)+accum_out for 3 (runs concurrently).
  phase 1.5: m = min(r, N-1-r); q, s; local num partial.
  phase 2: per u-subtile masks OQ=[q==Q] (128 wide), SS=[s>=S] (32 wide),
           weighted SSab, accumulated via PE into FH PSUM [128, 66].
  collective: AllReduce [128, 67] (F | H | num).
  phase 3: SufH via strict-lower-tri matmul, denom/ln on all 4096 windows,
           minus num total; identical scalar on every core.

`reps` replicates the whole body serially inside one NEFF -- used only for
slope-based wall-clock timing (NTFF profiling is unavailable here).
"""

import numpy as np

import concourse.bacc as bacc
import concourse.bass as bass
import concourse.mybir as mybir
import concourse.tile as tile

N = 8192
NCORE = 8
P = 128
US = N // NCORE          # 1024 u's per core
UT = US // P             # 8 u-subtiles per core
NPAIR = N // 2           # 4096 loss terms
NQ = 128                 # coarse window blocks (i = 32Q + S)
NS = 32
NCHUNK = 2               # j-dim chunks for DMA/compute pipelining
DVE_K = (0, 1, 2, 3, 4)  # u-subtiles ranked on the vector engine
ACT_K = (5, 6, 7)        # u-subtiles ranked on the scalar engine (sign trick)

F32 = mybir.dt.float32
BF16 = mybir.dt.bfloat16
I16 = mybir.dt.int16
AF = mybir.ActivationFunctionType
OP = mybir.AluOpType


def build_module(
    debug: bool = False,
    reps: int = 1,
    collective: bool = True,
    n1d: int = len(DVE_K),
    nchunk: int = NCHUNK,
    work_bufs: int = 2,
):
    dve_k = tuple(range(n1d))
    act_k = tuple(range(n1d, UT))
    # progressive chunk sizes: small first chunk so compares start early,
    # large later chunks to amortize per-instruction overhead
    if nchunk == 2:
        bounds = (0, 1024, 4096, N)
    else:
        bounds = tuple(q * (N // nchunk) for q in range(nchunk)) + (N,)
    nch = len(bounds) - 1
    nc = bacc.Bacc(
        "TRN2",
        target_bir_lowering=False,
        debug=False,
        enable_asserts=False,
        num_devices=NCORE,
    )

    t_row = nc.dram_tensor("t_row", [1, N], F32, kind="ExternalInput")
    # packed small consts: [tcol 8 | pcol 8] per partition
    NPK = 2 * UT
    packed = nc.dram_tensor("packed", [P, NPK], F32, kind="ExternalInput")
    out_part = nc.dram_tensor("out_part", [1, 1], F32, kind="ExternalOutput")
    if debug:
        dbg_r = nc.dram_tensor("dbg_r", [P, UT], F32, kind="ExternalOutput")
        dbg_m = nc.dram_tensor("dbg_m", [P, UT], F32, kind="ExternalOutput")
        dbg_fh = nc.dram_tensor("dbg_fh", [P, 67], F32, kind="ExternalOutput")

    with tile.TileContext(nc) as tc:
        with (
            tc.tile_pool(name="consts", bufs=1) as consts,
            tc.tile_pool(name="rep", bufs=1) as rp,
            tc.tile_pool(name="work", bufs=work_bufs) as work,
            tc.tile_pool(name="psum", bufs=1, space="PSUM") as psum,
            tc.tile_pool(name="dram", bufs=1, space="DRAM") as dram,
        ):
            # ---- constant/small loads ----
            packed_sb = consts.tile([P, NPK], F32)
            nc.scalar.dma_start(packed_sb[:], packed.ap())
            tcol_sb = packed_sb[:, 0:UT]
            pcol_sb = packed_sb[:, UT : 2 * UT]

            # on-device iotas (Pool engine, init-time only)
            iq_i = consts.tile([P, NQ], I16)
            nc.gpsimd.iota(iq_i[:], [[1, NQ]], base=0, channel_multiplier=0)
            is_i = consts.tile([P, NS], I16)
            nc.gpsimd.iota(is_i[:], [[1, NS]], base=0, channel_multiplier=0)
            ip_i = consts.tile([P, 1], I16)
            nc.gpsimd.iota(ip_i[:], [[1, 1]], base=0, channel_multiplier=1)
            iw_i = consts.tile([P, NS], I16)
            nc.gpsimd.iota(iw_i[:], [[-2, NS]], base=N, channel_multiplier=-64)

            iotaQ = consts.tile([P, NQ], F32)
            nc.vector.tensor_copy(iotaQ[:], iq_i[:])
            iotaS = consts.tile([P, NS], F32)
            nc.vector.tensor_copy(iotaS[:], is_i[:])
            # 32-grid thresholds 0,32,...,4096 and offset iota -32..-1
            i32_i = consts.tile([P, NQ + 1], I16)
            nc.gpsimd.iota(i32_i[:], [[32, NQ + 1]], base=0, channel_multiplier=0)
            iota32e = consts.tile([P, NQ + 1], F32)
            nc.vector.tensor_copy(iota32e[:], i32_i[:])
            iotaSm = consts.tile([P, NS], F32)
            nc.vector.tensor_scalar(iotaSm[:], iotaS[:], -32.0, None, OP.add)
            win_sb = consts.tile([P, NS], F32)
            nc.vector.tensor_copy(win_sb[:], iw_i[:])
            # strict lower triangle [p, Q] = [Q < p] for suffix sums
            ip_f = consts.tile([P, 1], F32)
            nc.vector.tensor_copy(ip_f[:], ip_i[:])
            tri = consts.tile([P, NQ], F32)
            nc.vector.tensor_scalar(tri[:], iotaQ[:], ip_f[:, 0:1], None, OP.is_lt)

            negt = consts.tile([P, UT], F32)
            nc.vector.tensor_scalar(negt[:], tcol_sb, -1.0, None, OP.mult)
            ones_col = consts.tile([P, 1], F32)
            nc.vector.memset(ones_col[:], 1.0)

            # own-slice weights: a = exp(p), b = exp(-p), f32 + bf16
            ab = consts.tile([P, UT, 2], F32)
            nc.scalar.activation(ab[:, :, 0], pcol_sb, AF.Exp)
            nc.scalar.activation(ab[:, :, 1], pcol_sb, AF.Exp, scale=-1.0)
            ab_bf = consts.tile([P, UT, 2], BF16)
            nc.vector.tensor_copy(ab_bf[:], ab[:])

            for _rep in range(reps):
                # ---- phase 1: ranks, chunked along j for DMA overlap ----
                racc_d = rp.tile([P, UT * nch], F32, tag="racc_d")
                racc_a = rp.tile([P, UT * nch], F32, tag="racc_a")
                for q in range(nch):
                    lo, hi = bounds[q], bounds[q + 1]
                    cw = hi - lo
                    tb = work.tile([P, cw], F32, tag=f"tb{q}")
                    nc.sync.dma_start(
                        tb[:],
                        t_row.ap()[0:1, lo:hi].to_broadcast((P, cw)),
                    )
                    scr_d = rp.tile([P, max(b - a for a, b in zip(bounds, bounds[1:]))],
                                    F32, tag="scr_d")
                    scr_a = rp.tile([P, max(b - a for a, b in zip(bounds, bounds[1:]))],
                                    F32, tag="scr_a")
                    for k in range(UT):
                        if k in dve_k:
                            col = racc_d[:, k * nch + q : k * nch + q + 1]
                            nc.vector.tensor_scalar(
                                scr_d[:, 0:cw], tb[:], tcol_sb[:, k : k + 1], None,
                                OP.is_gt, OP.add, accum_out=col,
                            )
                        else:
                            col = racc_a[:, k * nch + q : k * nch + q + 1]
                            nc.scalar.activation(
                                scr_a[:, 0:cw], tb[:], AF.Sign,
                                bias=negt[:, k : k + 1], accum_out=col,
                            )

                rsum = rp.tile([P, UT], F32, tag="rsum")
                nd = len(dve_k)
                nc.vector.tensor_reduce(
                    rsum[:, 0:nd],
                    racc_d[:, 0 : nd * nch].rearrange(
                        "p (k q) -> p k q", q=nch
                    ),
                    axis=mybir.AxisListType.X,
                    op=OP.add,
                )
                if act_k:
                    nc.vector.tensor_reduce(
                        rsum[:, nd:UT],
                        racc_a[:, nd * nch : UT * nch].rearrange(
                            "p (k q) -> p k q", q=nch
                        ),
                        axis=mybir.AxisListType.X,
                        op=OP.add,
                    )
                # ACT subtiles hold sum-of-sign: r = (s + N-1) / 2
                ka, kb = (min(act_k), max(act_k) + 1) if act_k else (0, 0)
                if act_k:
                    nc.vector.tensor_scalar(
                        rsum[:, ka:kb], rsum[:, ka:kb], float(N - 1), 0.5,
                        OP.add, OP.mult,
                    )
                if debug:
                    nc.sync.dma_start(dbg_r.ap(), rsum[:])

                # ---- phase 1.5: m, q, s, num partial ----
                tmp = rp.tile([P, UT], F32, tag="tmp")
                nc.vector.tensor_scalar(
                    tmp[:], rsum[:], float(N - 1), -1.0, OP.subtract, OP.mult
                )
                m_col = rp.tile([P, UT], F32, tag="m_col")
                nc.vector.tensor_tensor(m_col[:], rsum[:], tmp[:], OP.min)
                if debug:
                    nc.sync.dma_start(dbg_m.ap(), m_col[:])
                # step masks on the 32-grid; accum gives q+1 per subtile
                step = rp.tile([P, UT, NQ + 1], BF16, tag="step")
                qp1 = rp.tile([P, UT], F32, tag="qp1")
                for k in range(UT):
                    nc.vector.tensor_scalar(
                        step[:, k, :], iota32e[:], m_col[:, k : k + 1], None,
                        OP.is_le, OP.add, accum_out=qp1[:, k : k + 1],
                    )
                # s - 32 = m - 32*(q+1), compared against iotaSm (-32..-1)
                sm_col = rp.tile([P, UT], F32, tag="sm_col")
                nc.vector.scalar_tensor_tensor(
                    sm_col[:], qp1[:], -32.0, m_col[:], OP.mult, OP.add
                )

                sgn = rp.tile([P, UT], F32, tag="sgn")
                nc.vector.tensor_scalar(sgn[:], rsum[:], float(NPAIR), None, OP.is_lt)
                nc.vector.tensor_scalar(sgn[:], sgn[:], 2.0, -1.0, OP.mult, OP.add)
                xp = rp.tile([P, UT], F32, tag="xp")
                nc.vector.tensor_tensor(xp[:], sgn[:], pcol_sb, OP.mult)
                xq = rp.tile([P, 1], F32, tag="xq")
                nc.vector.tensor_reduce(
                    xq[:], xp[:], axis=mybir.AxisListType.X, op=OP.add
                )
                np_ps = psum.tile([1, 1], F32, tag="np_ps")
                nc.tensor.matmul(
                    np_ps[:], lhsT=xq[:], rhs=ones_col[:], start=True, stop=True
                )

                # ---- phase 2: partial F/H histograms via PE ----
                fh_ps = psum.tile([P, 66], F32, tag="fh_ps")
                for k in range(UT):
                    oq = work.tile([P, NQ], BF16, tag="oq")
                    nc.vector.tensor_tensor(
                        oq[:], step[:, k, 0:NQ], step[:, k, 1 : NQ + 1], OP.subtract
                    )
                    ss = work.tile([P, NS], BF16, tag="ss")
                    nc.vector.tensor_scalar(
                        ss[:], iotaSm[:], sm_col[:, k : k + 1], None, OP.is_le
                    )
                    ssab = work.tile([P, 2 * NS], BF16, tag="ssab")
                    nc.vector.tensor_scalar(
                        ssab[:, 0:NS], ss[:], ab[:, k, 0:1], None, OP.mult
                    )
                    nc.vector.tensor_scalar(
                        ssab[:, NS : 2 * NS], ss[:], ab[:, k, 1:2], None, OP.mult
                    )
                    nc.tensor.matmul(
                        fh_ps[:, 0:64], lhsT=oq[:], rhs=ssab[:],
                        start=(k == 0), stop=(k == UT - 1),
                    )
                    nc.tensor.matmul(
                        fh_ps[:, 64:66], lhsT=oq[:], rhs=ab_bf[:, k, :],
                        start=(k == 0), stop=(k == UT - 1),
                    )

                fh_in = rp.tile([P, 67], F32, tag="fh_in")
                nc.vector.tensor_copy(fh_in[:, 0:66], fh_ps[:])
                nc.vector.memset(fh_in[:, 66:67], 0.0)
                nc.vector.tensor_copy(fh_in[0:1, 66:67], np_ps[0:1, :])

                fh_dram = dram.tile([P, 67], F32, tag="fh_dram")
                nc.sync.dma_start(fh_dram[:], fh_in[:])
                fhall_dram = dram.tile([P, 67], F32, tag="fhall_dram")
                if collective:
                    nc.gpsimd.collective_compute(
                        "AllReduce",
                        OP.add,
                        replica_groups=[list(range(NCORE))],
                        ins=[fh_dram[:].opt()],
                        outs=[fhall_dram[:].opt()],
                    )
                else:  # timing-sim variant: stand-in DMA, wrong data, same shapes
                    nc.sync.dma_start(fhall_dram[:], fh_dram[:])
                fh_sb = rp.tile([P, 67], F32, tag="fh_sb")
                nc.sync.dma_start(fh_sb[:], fhall_dram[:])
                if debug:
                    nc.sync.dma_start(dbg_fh.ap(), fh_sb[:])

                # ---- phase 3: suffix sums, denom, ln, output ----
                suf_ps = psum.tile([P, 2], F32, tag="suf_ps")
                nc.tensor.matmul(
                    suf_ps[:], lhsT=tri[:], rhs=fh_sb[:, 64:66],
                    start=True, stop=True,
                )
                suf_sb = rp.tile([P, 2], F32, tag="suf_sb")
                nc.vector.tensor_copy(suf_sb[:], suf_ps[:])

                at = rp.tile([P, NS], F32, tag="at")
                nc.vector.tensor_scalar(
                    at[:], fh_sb[:, 0:NS], suf_sb[:, 0:1], None, OP.add
                )
                bt = rp.tile([P, NS], F32, tag="bt")
                nc.vector.tensor_scalar(
                    bt[:], fh_sb[:, NS : 2 * NS], suf_sb[:, 1:2], None, OP.add
                )
                den = rp.tile([P, NS], F32, tag="den")
                nc.vector.tensor_tensor(den[:], at[:], bt[:], OP.mult)
                nc.vector.tensor_tensor(den[:], den[:], win_sb[:], OP.subtract)
                nc.vector.tensor_scalar(den[:], den[:], 1e-8, None, OP.max)
                logd = rp.tile([P, NS], F32, tag="logd")
                lnacc = rp.tile([P, 1], F32, tag="lnacc")
                nc.scalar.activation(logd[:], den[:], AF.Ln, accum_out=lnacc[:])
                ln_ps = psum.tile([1, 1], F32, tag="ln_ps")
                nc.tensor.matmul(
                    ln_ps[:], lhsT=lnacc[:], rhs=ones_col[:], start=True, stop=True
                )
                out_sb = rp.tile([1, 1], F32, tag="out_sb")
                nc.vector.tensor_tensor(
                    out_sb[:], ln_ps[0:1, :], fh_sb[0:1, 66:67], OP.subtract
                )
                nc.sync.dma_start(out_part.ap(), out_sb[:])

    nc.compile()
    return nc


def make_in_maps(pred: np.ndarray, target: np.ndarray):
    pred = np.ascontiguousarray(pred, dtype=np.float32).reshape(N)
    target = np.ascontiguousarray(target, dtype=np.float32).reshape(N)
    t_row = target.reshape(1, N)
    in_maps = []
    for c in range(NCORE):
        tsl = target[c * US : (c + 1) * US]
        psl = pred[c * US : (c + 1) * US]
        pk = np.concatenate(
            [
                tsl.reshape(UT, P).T,
                psl.reshape(UT, P).T,
            ],
            axis=1,
        ).astype(np.float32)
        in_maps.append(
            {
                "t_row": t_row,
                "packed": np.ascontiguousarray(pk),
            }
        )
    return in_maps


_CACHE = {}


def _get_module():
    if "nc" not in _CACHE:
        _CACHE["nc"] = build_module(debug=False)
    return _CACHE["nc"]


def kernel(pred: np.ndarray, target: np.ndarray) -> np.ndarray:
    from concourse import bass_utils

    nc = _get_module()
    in_maps = make_in_maps(pred, target)
    res = bass_utils.run_bass_kernel_spmd(nc, in_maps, core_ids=list(range(NCORE)))
    return np.asarray(res.results[0]["out_part"][0, 0], dtype=np.float32)
